# revision 40
# baseline (speedup 1.0000x reference)
"""Trainium2 Bass kernel for nn_LinearAttentionBlock (linear attention).

Per-core (data-parallel over batch, 1 batch / core):
  x_b [4096, 512] -> qkv = x_b @ w_qkv -> per-head LayerNorm(q), LayerNorm(k)
  dots_h = LN(k)_h^T @ v_h   [64, 64]
  out_h  = LN(q)_h @ dots_h / 4096
  out    = concat_h(out_h)   [4096, 512]

Fast path (q/k gamma == 1, beta == 0 — checked at runtime in kernel()):
  - Weights are column-centered per head ON HOST, so q/k come out of the
    qkv matmul already mean-subtracted (LayerNorm mean folded into weights),
    and w/x are pre-cast to bf16 host-side (halves input DMA, no on-device
    weight prep).
  - x^T tiles come from batched DRAM->SBUF DMA-transposes (one per 4-tile
    chunk, sync HWDGE queue); q-hat^T via batched SBUF transposes on the
    scalar HWDGE queue.
  - LN variance: ACT squares (bf16), DVE segmented reduce + reciprocal,
    DVE rstd applies; v copy on the Pool engine to balance ACT.
  - dots accumulated in one PSUM bank (4 head-pair blocks side by side,
    single accumulation group); out = pair-blockdiag matmul of q-hat^T,
    stored to DRAM as bf16 DIRECTLY FROM PSUM (dge-cast), upcast on host.

General path (arbitrary gamma/beta): previous proven kernel, kept intact.
"""
import threading

import numpy as np

import concourse.bacc as bacc
import concourse.bass as bass
import concourse.mybir as mybir
from concourse.tile import TileContext
from concourse.tile_rust import add_dep_helper

P = 128
NTOK = 4096          # tokens per batch (64*64)
CIN = 512            # input channels
N3 = 3 * CIN         # qkv columns
MT = NTOK // P       # 32 m-tiles
KC = CIN // P        # 4 k-chunks
H = 8                # heads
D = 64               # dim per head
NPAIR = H // 2       # 4 head pairs
CH = 4               # m-tiles per chunk
NCH = MT // CH       # 8 chunks
NCORES = 8
LN_EPS = 1e-5

f32 = mybir.dt.float32
bf16 = mybir.dt.bfloat16
X = mybir.AxisListType.X
MUL = mybir.AluOpType.mult
SUB = mybir.AluOpType.subtract
ADD = mybir.AluOpType.add
COPY = mybir.ActivationFunctionType.Copy
SQUARE = mybir.ActivationFunctionType.Square
SQRT = mybir.ActivationFunctionType.Sqrt


def _bc(ap, n):
    """Append a stride-0 broadcast dim of size n to an AP."""
    return bass.AP(ap.tensor, ap.offset, list(ap.ap) + [[0, n]])


def _col64(dram_ap):
    """View a [64] DRAM tensor as a [64, 1] column AP (partition-major)."""
    return bass.AP(dram_ap.tensor, dram_ap.offset, [[1, D], [1, 1]])


# ---------------------------------------------------------------------------
# Fast path: trivial affine (gamma == 1, beta == 0), bf16 xT/w, bf16 out.
# ---------------------------------------------------------------------------

def _act_rsqrt(nc, out, in_, bias_ap):
    """Raw Rsqrt activation (bass blocks it by default; our 2e-2 tolerance
    is far above the pwp table's error)."""
    eng = nc.scalar
    inputs = [
        eng.lower_ap(in_),
        eng.lower_ap(bias_ap),
        mybir.ImmediateValue(dtype=f32, value=1.0),   # scale
        mybir.ImmediateValue(dtype=f32, value=0.0),   # alpha
    ]
    return eng.add_instruction(mybir.InstActivation(
        name=eng.bass.get_next_instruction_name(),
        func=mybir.ActivationFunctionType.Rsqrt,
        ins=inputs,
        outs=[eng.lower_ap(out)]))


def _body_fast(nc, tc, x, w, out):
    with tc.tile_pool(name="singles", bufs=1) as singles, \
         tc.tile_pool(name="xTp", bufs=4) as xTp, \
         tc.tile_pool(name="sqp", bufs=2) as sqp, \
         tc.tile_pool(name="stp", bufs=3) as stp, \
         tc.tile_pool(name="kvp", bufs=3) as kvp:

        # ---- P0: weight load, split per c-chunk so matmuls start early ----
        # (host already centered q/k columns and cast to bf16)
        w_bf = singles.tile([P, KC, N3], bf16)
        for c in range(KC):
            nc.scalar.dma_start(out=w_bf[:, c, :],
                                in_=w[c * P:(c + 1) * P, :])

        eps_t = singles.tile([P, 1], f32)
        nc.vector.memset(eps_t[:], float(D) * LN_EPS)
        d_all = singles.tile([P, NPAIR, P], bf16)
        nc.vector.memset(d_all[:], 0.0)

        # q-hat/k-hat store, g-major: [p, {q,k}, tile, cin]
        qkhat = singles.tile([P, 2, MT, CIN], bf16)
        # qhatT layout: [p, tile, c, tok] so a per-chunk DMA transpose of
        # the q half of qkhat lands with a mergeable 3D output AP.
        qhatT = singles.tile([P, MT, KC, P], bf16)

        with tc.tile_pool(name="ps_acc", bufs=1, space="PSUM") as ps_acc:
            dots_ps = ps_acc.tile([P, 4 * P], f32)
            with tc.tile_pool(name="ps_qk", bufs=3, space="PSUM") as ps_qk, \
                 tc.tile_pool(name="ps_v", bufs=1, space="PSUM") as ps_v:
                # guards against PSUM-reset racing the previous tile's
                # readers: a matmul group's write is tracked at its stop, so
                # the start=True reset can otherwise slip ahead
                prev_vcopy = [None]
                prev_mult = [None, None, None]
                for ci in range(NCH):
                    # x^T chunk load; host layout [ci, p, c, n] makes each
                    # partition line one contiguous 4KB run. First chunks go
                    # on the sync HWDGE queue (gpsimd's SWDGE starts slowly).
                    xT = xTp.tile([P, KC, CH * P], bf16)
                    xeng = nc.sync if ci < 2 else nc.gpsimd
                    xeng.dma_start(
                        out=xT[:],
                        in_=x[ci * P:(ci + 1) * P, :].rearrange(
                            "p (c n) -> p c n", c=KC))
                    for tt in range(CH):
                        mt = ci * CH + tt
                        tok = slice(tt * P, (tt + 1) * P)

                        qk_ps = ps_qk.tile([P, 2, CIN], f32, tag="qk")
                        v_ps = ps_v.tile([P, CIN], f32, tag="v")
                        # chunk-outer so lhsT repeats for q/k/v back-to-back
                        for c in range(KC):
                            for nb, pst in enumerate(
                                    (qk_ps[:, 0, :], qk_ps[:, 1, :], v_ps[:])):
                                mm = nc.tensor.matmul(
                                    pst, lhsT=xT[:, c, tok],
                                    rhs=w_bf[:, c, nb * CIN:(nb + 1) * CIN],
                                    start=(c == 0), stop=(c == KC - 1))
                                if c == 0 and nb == 2 and prev_vcopy[0] is not None:
                                    add_dep_helper(mm.ins, prev_vcopy[0].ins,
                                                   sync=True,
                                                   reason="psum v reset WAR")
                                if c == 0 and nb == 0 and prev_mult[mt % 3] is not None:
                                    add_dep_helper(mm.ins, prev_mult[mt % 3].ins,
                                                   sync=True,
                                                   reason="psum qk reset WAR")

                        # v copy first: ps_v is single-buffered, so the next
                        # tile's v matmuls wait on this
                        v_bf = kvp.tile([P, CIN], bf16, tag="v_bf")
                        prev_vcopy[0] = nc.scalar.copy(v_bf[:], v_ps[:])

                        # LN stats: square (ACT), segmented sum (DVE), rsqrt
                        sq2 = sqp.tile([P, 2, CIN], bf16, tag="sq")
                        nc.scalar.square(sq2[:], qk_ps[:])
                        st = stp.tile([P, 2, H], f32, tag="st")
                        nc.vector.reduce_sum(
                            st[:],
                            sq2.rearrange("p g (h d) -> p g h d", d=D), axis=X)
                        rstd = stp.tile([P, 2, H], f32, tag="rstd")
                        _act_rsqrt(nc, rstd[:], st[:], eps_t[:])

                        # apply rstd to q and k in one DVE pass
                        prev_mult[mt % 3] = nc.vector.tensor_tensor(
                            out=qkhat[:, :, mt, :].rearrange(
                                "p g (h d) -> p g h d", d=D),
                            in0=qk_ps.rearrange("p g (h d) -> p g h d", d=D),
                            in1=_bc(rstd[:], D), op=MUL)

                        # dots: 4 pair blocks in one bank, one accum group
                        mm0 = None
                        for pr in range(NPAIR):
                            mm = nc.tensor.matmul(
                                dots_ps[:, pr * P:(pr + 1) * P],
                                lhsT=qkhat[:, 1, mt, pr * P:(pr + 1) * P],
                                rhs=v_bf[:, pr * P:(pr + 1) * P],
                                start=(mt == 0 and pr == 0),
                                stop=(mt == MT - 1 and pr == NPAIR - 1))
                            if mt == 0:
                                if pr == 0:
                                    mm0 = mm
                                else:
                                    add_dep_helper(mm.ins, mm0.ins, sync=False,
                                                   reason="psum group order")

                        # q-hat transpose per tile; alternate HWDGE queues
                        eng = nc.sync if mt % 2 == 0 else nc.scalar
                        eng.dma_start(
                            out=qhatT[:, mt, :, :],
                            in_=qkhat[:, 0, mt, :],
                            transpose=True)

            # -------- P2: dots -> d_all (pair blockdiag, scaled) --------
            # rstd was computed from sum(q~^2) (without /D), so rstd here is
            # sqrt(D) too small; q-hat and k-hat each carry 1/sqrt(D) -> the
            # dots fixup multiplies by D; plus the final 1/NTOK.
            for half in (0, 1):
                sl = slice(half * D, (half + 1) * D)
                nc.scalar.activation(
                    out=d_all[sl, :, half * D:(half + 1) * D],
                    in_=dots_ps[sl, :].rearrange("p (pr x) -> p pr x", x=P)[
                        :, :, half * D:(half + 1) * D],
                    func=COPY, scale=float(D) / NTOK)

        # ---- P3: out = qhat @ D (pair blockdiag), bf16 staged, chunk DMA ---
        with tc.tile_pool(name="ps_out", bufs=4, space="PSUM") as ps_out, \
             tc.tile_pool(name="outp", bufs=3) as outp:
            # same PSUM-reset WAR guard as in P1: the start=True reset of a
            # recycled o_ps slot must wait for both half-copies of the tile
            # that used it 4 iterations ago
            prev_copies = [None] * 4
            for ci in range(NCH):
                out_ch = outp.tile([P, CH, CIN], bf16)
                for tt in range(CH):
                    nt = ci * CH + tt
                    o_ps = ps_out.tile([P, CIN], f32, tag="o")
                    mm0 = None
                    for pr in range(NPAIR):
                        mm = nc.tensor.matmul(
                            o_ps[:, pr * P:(pr + 1) * P],
                            lhsT=qhatT[:, nt, pr, :],
                            rhs=d_all[:, pr, :],
                            start=(pr == 0), stop=(pr == NPAIR - 1))
                        if pr == 0:
                            mm0 = mm
                            if prev_copies[nt % 4] is not None:
                                for pc in prev_copies[nt % 4]:
                                    add_dep_helper(mm.ins, pc.ins, sync=True,
                                                   reason="psum o reset WAR")
                        else:
                            add_dep_helper(mm.ins, mm0.ins, sync=False,
                                           reason="psum group start order")
                    # split each PSUM->SBUF copy across ACT and DVE
                    c1 = nc.scalar.copy(out_ch[:, tt, 0:CIN // 2],
                                        o_ps[:, 0:CIN // 2])
                    c2 = nc.vector.tensor_copy(out_ch[:, tt, CIN // 2:],
                                               o_ps[:, CIN // 2:])
                    prev_copies[nt % 4] = (c1, c2)
                # out is partition-major in DRAM ([p, tile, k]); host
                # un-permutes. Contiguous 4KB runs on both sides. Alternate
                # the two fast HWDGE queues so stores run in parallel.
                eng = nc.scalar if ci % 2 == 0 else nc.sync
                eng.dma_start(
                    out=out[:, ci * CH:(ci + 1) * CH, :], in_=out_ch[:])


def build_kernel_fast():
    nc = bacc.Bacc(None, target_bir_lowering=False)
    # x is fed host-permuted: [ci, p, c, n] flattened to 2D
    x = nc.declare_dram_parameter("x", [NCH * P, KC * CH * P], bf16,
                                  isOutput=False)[:, :]
    w = nc.declare_dram_parameter("w_qkv", [CIN, N3], bf16, isOutput=False)[:, :]
    # out is partition-major [p, tile, k]; host un-permutes
    out = nc.declare_dram_parameter("out", [P, MT, CIN], bf16,
                                    isOutput=True)[:, :, :]
    with TileContext(nc) as tc:
        _body_fast(nc, tc, x, w, out)
    nc.compile()
    return nc


# ---------------------------------------------------------------------------
# General path: arbitrary gamma/beta (previous proven kernel, f32 inputs).
# ---------------------------------------------------------------------------

def _body_general(nc, tc, pools, x, w, gq, bq, gk, bk, out):
    singles, xch, xTp, sqp, stp, kvp, outp = pools

    # ---------------- P0: weight prep ----------------
    w_f32 = singles.tile([P, KC, N3], f32)
    nc.sync.dma_start(out=w_f32[:], in_=w.rearrange("(c p) n -> p c n", p=P))

    wbar = singles.tile([P, KC, 2, H], f32)
    for part in (0, 1):
        nc.vector.reduce_sum(
            wbar[:, :, part, :],
            w_f32[:, :, part * CIN:(part + 1) * CIN].rearrange(
                "p c (h d) -> p c h d", d=D),
            axis=X)
    nc.vector.tensor_scalar_mul(out=wbar[:], in0=wbar[:], scalar1=1.0 / D)

    w_bf = singles.tile([P, KC, N3], bf16)
    for part in (0, 1):
        nc.vector.tensor_tensor(
            out=w_bf[:, :, part * CIN:(part + 1) * CIN].rearrange(
                "p c (h d) -> p c h d", d=D),
            in0=w_f32[:, :, part * CIN:(part + 1) * CIN].rearrange(
                "p c (h d) -> p c h d", d=D),
            in1=_bc(wbar[:, :, part, :], D),
            op=SUB)
    nc.vector.tensor_copy(out=w_bf[:, :, 2 * CIN:], in_=w_f32[:, :, 2 * CIN:])

    # gamma/beta columns replicated onto both partition halves
    gq2 = singles.tile([P, 1], f32)
    gk2 = singles.tile([P, 1], f32)
    bk2 = singles.tile([P, 1], f32)
    for half in (0, 1):
        sl = slice(half * D, (half + 1) * D)
        nc.sync.dma_start(out=gq2[sl, :], in_=_col64(gq))
        nc.sync.dma_start(out=gk2[sl, :], in_=_col64(gk))
        nc.sync.dma_start(out=bk2[sl, :], in_=_col64(bk))
    bq_bf = singles.tile([D, 1], bf16)
    nc.gpsimd.dma_start(out=bq_bf[:], in_=_col64(bq))

    eps_t = singles.tile([P, 1], f32)
    nc.vector.memset(eps_t[:], float(D) * LN_EPS)
    ones_bf = singles.tile([P, P], bf16)
    nc.vector.memset(ones_bf[:], 1.0)

    qhat_store = singles.tile([P, MT, CIN], bf16)
    qhatT = singles.tile([P, KC, NTOK], bf16)

    with tc.tile_pool(name="ps_acc", bufs=1, space="PSUM") as ps_acc:
        dots_ps = ps_acc.tile([P, 4 * P], f32)
        sumv_ps = ps_acc.tile([P, CIN], f32)
        with tc.tile_pool(name="ps_qkv", bufs=2, space="PSUM") as ps_qkv:
            _p1_loop_general(nc, x, w_bf, eps_t, ones_bf, qhat_store, qhatT,
                             dots_ps, sumv_ps,
                             (xch, xTp, sqp, stp, kvp, ps_qkv))

        # ---------------- P2: dots fixups ----------------
        dots_sb = singles.tile([P, 4 * P], f32)
        nc.vector.tensor_copy(out=dots_sb[:], in_=dots_ps[:])
        sumv_sb = singles.tile([P, CIN], f32)
        nc.vector.tensor_copy(out=sumv_sb[:], in_=sumv_ps[:])

    ktmp = singles.tile([P, NPAIR, D], f32)
    bsum = singles.tile([P, NPAIR, D], f32)
    deo = singles.tile([P, NPAIR, D], f32)
    for half in (0, 1):
        sl = slice(half * D, (half + 1) * D)
        # KV diag block, scaled by gamma_k * 8
        nc.vector.tensor_scalar(
            out=ktmp[sl, :, :],
            in0=dots_sb[sl, :].rearrange("p (pr x) -> p pr x", x=P)[
                :, :, half * D:(half + 1) * D],
            scalar1=gk2[sl, :], scalar2=8.0, op0=MUL, op1=MUL)
        # beta_k (x) sumV
        nc.vector.tensor_scalar(
            out=bsum[sl, :, :],
            in0=sumv_sb[sl, :].rearrange(
                "p (pr two d) -> p pr two d", two=2, d=D)[:, :, half, :],
            scalar1=bk2[sl, :], scalar2=None, op0=MUL)
    nc.vector.tensor_add(deo[:], ktmp[:], bsum[:])

    d_all = singles.tile([P, NPAIR, P], bf16)
    nc.vector.memset(d_all[:], 0.0)
    for half in (0, 1):
        sl = slice(half * D, (half + 1) * D)
        nc.vector.tensor_scalar(
            out=d_all[sl, :, half * D:(half + 1) * D],
            in0=deo[sl, :, :],
            scalar1=gq2[sl, :], scalar2=8.0 / NTOK, op0=MUL, op1=MUL)

    # c row: beta_q @ dots / NTOK, replicated over partitions
    dstack = singles.tile([D, H, D], bf16)
    nc.vector.tensor_copy(
        out=dstack.rearrange("p (pr two) d -> p pr two d", two=2)[:, :, 0, :],
        in_=deo[0:D, :, :])
    nc.gpsimd.dma_start(
        out=dstack.rearrange("p (pr two) d -> p pr two d", two=2)[:, :, 1, :],
        in_=deo[D:P, :, :])

    with tc.tile_pool(name="ps_fix", bufs=1, space="PSUM") as ps_fix, \
         tc.tile_pool(name="ps_out", bufs=2, space="PSUM") as ps_out:
        c_ps = ps_fix.tile([1, CIN], f32)
        nc.tensor.matmul(c_ps[:], lhsT=bq_bf[:],
                         rhs=dstack.rearrange("p h d -> p (h d)"),
                         start=True, stop=True)
        c_bf = singles.tile([1, CIN], bf16)
        nc.vector.tensor_scalar_mul(out=c_bf[:], in0=c_ps[:],
                                    scalar1=1.0 / NTOK)
        crep_ps = ps_fix.tile([P, CIN], f32)
        nc.tensor.matmul(crep_ps[:], lhsT=ones_bf[0:1, :], rhs=c_bf[:],
                         start=True, stop=True)
        crep = singles.tile([P, CIN], f32)
        nc.vector.tensor_copy(out=crep[:], in_=crep_ps[:])

        # ------------ P3: out = qhat @ D (pair blockdiag) + c ------------
        for ci in range(MT // CH):
            out_ch = outp.tile([P, CH, CIN], f32)
            for tt in range(CH):
                nt = ci * CH + tt
                o_ps = ps_out.tile([P, CIN], f32, tag="o")
                mm0 = None
                for pr in range(NPAIR):
                    mm = nc.tensor.matmul(
                        o_ps[:, pr * P:(pr + 1) * P],
                        lhsT=qhatT[:, pr, nt * P:(nt + 1) * P],
                        rhs=d_all[:, pr, :],
                        start=(pr == 0), stop=(pr == NPAIR - 1))
                    if pr == 0:
                        mm0 = mm
                    else:
                        add_dep_helper(mm.ins, mm0.ins, sync=False,
                                       reason="psum group start order")
                nc.vector.tensor_tensor(
                    out=out_ch[:, tt, :], in0=o_ps[:], in1=crep[:], op=ADD)
            nc.sync.dma_start(
                out=out[ci * CH * P:(ci + 1) * CH * P, :].rearrange(
                    "(t p) k -> p t k", p=P),
                in_=out_ch[:])


def _p1_loop_general(nc, x, w_bf, eps_t, ones_bf, qhat_store, qhatT,
                     dots_ps, sumv_ps, pools):
    xch, xTp, sqp, stp, kvp, ps_qkv = pools
    for ci in range(MT // CH):
        x_ch = xch.tile([P, CH, CIN], bf16)
        nc.gpsimd.dma_start(
            out=x_ch[:],
            in_=x[ci * CH * P:(ci + 1) * CH * P, :].rearrange(
                "(t p) k -> p t k", p=P))
        for tt in range(CH):
            mt = ci * CH + tt
            xT = xTp.tile([P, KC, P], bf16)
            nc.sync.dma_start(out=xT[:], in_=x_ch[:, tt, :], transpose=True)

            q_ps = ps_qkv.tile([P, CIN], f32, tag="q")
            k_ps = ps_qkv.tile([P, CIN], f32, tag="k")
            v_ps = ps_qkv.tile([P, CIN], f32, tag="v")
            for nb, pst in enumerate((q_ps, k_ps, v_ps)):
                for c in range(KC):
                    nc.tensor.matmul(
                        pst[:], lhsT=xT[:, c, :],
                        rhs=w_bf[:, c, nb * CIN:(nb + 1) * CIN],
                        start=(c == 0), stop=(c == KC - 1))

            # LN stats: squares (ACT), segmented sums (DVE)
            sq_q = sqp.tile([P, CIN], f32, tag="sq_q")
            sq_k = sqp.tile([P, CIN], f32, tag="sq_k")
            nc.scalar.square(sq_q[:], q_ps[:])
            nc.scalar.square(sq_k[:], k_ps[:])
            st = stp.tile([P, 2, H], f32, tag="st")
            nc.vector.reduce_sum(
                st[:, 0, :], sq_q.rearrange("p (h d) -> p h d", d=D), axis=X)
            nc.vector.reduce_sum(
                st[:, 1, :], sq_k.rearrange("p (h d) -> p h d", d=D), axis=X)
            rstd = stp.tile([P, 2, H], f32, tag="rstd")
            nc.scalar.activation(
                out=rstd[:], in_=st[:],
                func=mybir.ActivationFunctionType.Sqrt,
                bias=eps_t[:], scale=1.0)
            nc.vector.reciprocal(rstd[:], rstd[:])

            # apply rstd (x8 factor folded into D fixup)
            nc.vector.tensor_tensor(
                out=qhat_store[:, mt, :].rearrange("p (h d) -> p h d", d=D),
                in0=q_ps.rearrange("p (h d) -> p h d", d=D),
                in1=_bc(rstd[:, 0, :], D), op=MUL)
            khat = kvp.tile([P, CIN], bf16, tag="khat")
            nc.vector.tensor_tensor(
                out=khat.rearrange("p (h d) -> p h d", d=D),
                in0=k_ps.rearrange("p (h d) -> p h d", d=D),
                in1=_bc(rstd[:, 1, :], D), op=MUL)
            v_bf = kvp.tile([P, CIN], bf16, tag="v_bf")
            nc.scalar.copy(v_bf[:], v_ps[:])

            # stage 2: dots (4 pair blocks in one bank) + sumV
            mm0 = None
            for pr in range(NPAIR):
                mm = nc.tensor.matmul(
                    dots_ps[:, pr * P:(pr + 1) * P],
                    lhsT=khat[:, pr * P:(pr + 1) * P],
                    rhs=v_bf[:, pr * P:(pr + 1) * P],
                    start=(mt == 0 and pr == 0),
                    stop=(mt == MT - 1 and pr == NPAIR - 1))
                if mt == 0:
                    if pr == 0:
                        mm0 = mm
                    else:
                        add_dep_helper(mm.ins, mm0.ins, sync=False,
                                       reason="psum group start order")
            nc.tensor.matmul(sumv_ps[:], lhsT=ones_bf[:], rhs=v_bf[:],
                             start=(mt == 0), stop=(mt == MT - 1))

            # q-hat transpose into [c, n] layout
            nc.sync.dma_start(
                out=qhatT[:, :, mt * P:(mt + 1) * P],
                in_=qhat_store[:, mt, :], transpose=True)


def build_kernel_general():
    nc = bacc.Bacc(None, target_bir_lowering=False)
    x = nc.declare_dram_parameter("x", [NTOK, CIN], f32, isOutput=False)[:, :]
    w = nc.declare_dram_parameter("w_qkv", [CIN, N3], f32, isOutput=False)[:, :]
    gq = nc.declare_dram_parameter("q_gamma", [D], f32, isOutput=False)[:]
    bq = nc.declare_dram_parameter("q_beta", [D], f32, isOutput=False)[:]
    gk = nc.declare_dram_parameter("k_gamma", [D], f32, isOutput=False)[:]
    bk = nc.declare_dram_parameter("k_beta", [D], f32, isOutput=False)[:]
    out = nc.declare_dram_parameter("out", [NTOK, CIN], f32, isOutput=True)[:, :]

    with TileContext(nc) as tc:
        with tc.tile_pool(name="singles", bufs=1) as singles, \
             tc.tile_pool(name="xch", bufs=2) as xch, \
             tc.tile_pool(name="xTp", bufs=3) as xTp, \
             tc.tile_pool(name="sqp", bufs=2) as sqp, \
             tc.tile_pool(name="stp", bufs=3) as stp, \
             tc.tile_pool(name="kvp", bufs=3) as kvp, \
             tc.tile_pool(name="outp", bufs=2) as outp:
            pools = (singles, xch, xTp, sqp, stp, kvp, outp)
            _body_general(nc, tc, pools, x, w, gq, bq, gk, bk, out)
    nc.compile()
    return nc


def _enable_ldw_opt():
    """walrus elides redundant LDWEIGHTS for back-to-back matmuls sharing
    lhsT only with --enable-ldw-opt=true; concourse hardcodes false."""
    import concourse.bass_utils as bu

    if getattr(bu, "_ldw_opt_patched", False):
        return
    orig = bu.run_command

    def patched(cmd, **kw):
        cmd = ["--enable-ldw-opt=true" if c == "--enable-ldw-opt=false" else c
               for c in cmd]
        return orig(cmd, **kw)

    bu.run_command = patched
    bu._ldw_opt_patched = True


_LOCK = threading.Lock()
_CACHED = {}


def _get_nc(variant="fast"):
    with _LOCK:
        if variant not in _CACHED:
            _CACHED[variant] = (build_kernel_fast() if variant == "fast"
                                else build_kernel_general())
    return _CACHED[variant]


def _center_w(w_qkv):
    """Fold the per-head LayerNorm mean subtraction into the q/k weight
    columns (exact: LN mean of x@W equals x@(per-head col-centered W))."""
    w = np.asarray(w_qkv, dtype=np.float64).copy()
    for part in range(2):  # q and k blocks; v untouched
        blk = w[:, part * CIN:(part + 1) * CIN].reshape(CIN, H, D)
        blk -= blk.mean(axis=2, keepdims=True)
    return w.astype(np.float32)


def _is_trivial_affine(gq, bq, gk, bk):
    return (np.all(np.asarray(gq) == 1.0) and np.all(np.asarray(gk) == 1.0)
            and not np.any(np.asarray(bq)) and not np.any(np.asarray(bk)))


def make_in_maps(x, w_qkv, q_gamma, q_beta, k_gamma, k_beta, variant="fast"):
    x = np.asarray(x, dtype=np.float32)
    B = x.shape[0]
    if variant == "fast":
        import ml_dtypes
        w_bf = _center_w(w_qkv).astype(ml_dtypes.bfloat16)
        xs = x.reshape(B, NTOK, CIN).astype(ml_dtypes.bfloat16)
        in_maps = []
        for b in range(NCORES):
            # [ci, n, c, p] -> [ci, p, c, n]: per-partition 4KB runs on device
            xp = xs[b].reshape(NCH, CH * P, KC, P).transpose(0, 3, 2, 1)
            in_maps.append({
                "x": np.ascontiguousarray(xp).reshape(NCH * P, KC * CH * P),
                "w_qkv": w_bf,
            })
        return in_maps
    w = np.asarray(w_qkv, dtype=np.float32)
    return [{
        "x": np.ascontiguousarray(x[b].reshape(NTOK, CIN)),
        "w_qkv": w,
        "q_gamma": np.asarray(q_gamma, dtype=np.float32),
        "q_beta": np.asarray(q_beta, dtype=np.float32),
        "k_gamma": np.asarray(k_gamma, dtype=np.float32),
        "k_beta": np.asarray(k_beta, dtype=np.float32),
    } for b in range(NCORES)]


def kernel(x, w_qkv, q_gamma, q_beta, k_gamma, k_beta):
    from concourse.bass_utils import run_bass_kernel_spmd

    x = np.asarray(x, dtype=np.float32)
    B, L, W, C = x.shape
    variant = ("fast" if _is_trivial_affine(q_gamma, q_beta, k_gamma, k_beta)
               else "general")
    nc = _get_nc(variant)
    in_maps = make_in_maps(x, w_qkv, q_gamma, q_beta, k_gamma, k_beta,
                           variant=variant)
    res = run_bass_kernel_spmd(nc, in_maps, list(range(NCORES)))
    outs = []
    for b in range(NCORES):
        o = np.asarray(res.results[b]["out"])
        if variant == "fast":
            # [p, tile, k] -> [tile, p, k] -> [ntok, k]
            o = o.transpose(1, 0, 2).reshape(NTOK, H * D)
        outs.append(o)
    out = np.stack(outs)
    return out.reshape(B, L, W, H * D).astype(np.float32)
